# revision 2
# baseline (speedup 1.0000x reference)
"""Trainium2 Bass kernel for PVT-style spatial-reduction cross-attention.

Problem shapes (hardcoded): x [4, 3136, 512], v [4, 3136, 512], h=w=56,
8 heads (dh=64), sr_ratio=2 -> 784 kv tokens, fp32 I/O.

Sharding: 8 cores = 4 batches x 2 query-token halves. Each core computes
the full conv+LN+kv path for its batch (duplicated within the pair) and
attention + output projection for its 1568 query tokens. No collectives.

Layout strategy: activations are kept channel-major ("transposed") on chip
so every matmul contracts over the partition dim. The host supplies x^T and
v^T (layout choice during sharding). Scores are computed transposed
([ktok, qtok]); softmax denominators come from a ones-column appended to
the value matrix; normalization is deferred through the attention-output
matmul and applied via a gpsimd partition-broadcast of 1/denom.

Performance structure (v1 345us -> v2 205us -> this):
 - inputs host-packed into one [128, F] DRAM tensor per logical tensor and
   loaded with ONE dma_start each (~9 total): the Sync engine serializes
   dma_start issue at ~600ns each, so the old ~51-DMA startup was
   issue-bound, not bandwidth-bound.
 - biases come in as single [1, F] rows and are partition-broadcast on
   chip by gpsimd during the startup DMA window.
 - q projection runs in fp8e4 DoubleRow perf mode (2 rows/cycle): host
   quantizes x^T and q_w with fixed scales; descale is folded into the
   PSUM->SBUF evacuation multiply. q-path quantization noise is softmax-
   attenuated (scores shift by <1%), unlike the v path which must stay
   bf16 (attention output is a near-uniform average over 784 tokens, so
   iid per-token noise survives at full relative strength).
 - LayerNorm rstd = exp(-0.5*ln(var+eps)): Ln/Exp/Square share one ACT
   table set, while Sqrt forced ~7 ACT_TABLE_LOADs (~9us) interleaved
   with the exp stream.
 - query chunks are 4x392 (not 512,512,512,32): exp instructions are
   uniform [112, 784]-free; the old 32-wide tail spent ~200ns/instr ACT
   overhead on 56 tiny instructions.
 - phases fused per kv-chunk: conv -> LN -> transpose -> vv projection;
   kT projection in two batched halves; attention head pairs (2p, 2p+1)
   share kT/qT tiles at partition offsets 0/64 so their score matmuls
   occupy disjoint PE row groups and run concurrently; one ACT exp
   instruction covers both heads' scores (the exp stream on the Scalar
   engine is the kernel's second-longest engine load).
 - the 16 (q-chunk, head-pair) blocks form one flat software-pipelined
   stream (scores one block ahead of attention); output projection runs
   from a full-width [128, 1568] accumulator in 13 query chunks of 128,
   decoupled from the 392-query attention chunking.
 - all phase-4 SBUF-shuffle/output DMAs are deferred two pipeline blocks:
   dma_start semaphore waits execute in-order ON the Sync engine, and an
   early-issued DMA whose producer has not retired stalls every later
   cross-engine notify behind it.
"""

import functools
import sys

import numpy as np

try:
    import concourse.bass as bass
except ImportError:  # pragma: no cover
    sys.path.insert(0, "/opt/trn_rl_repo")
    import concourse.bass as bass

import ml_dtypes
from concourse import bass_utils, mybir
from concourse.masks import make_identity
from concourse.tile import TileContext

BF16 = mybir.dt.bfloat16
F32 = mybir.dt.float32
FP8 = mybir.dt.float8e4
NPBF = ml_dtypes.bfloat16
NPF8 = ml_dtypes.float8_e4m3

P = 128
C = 512          # channels
NH = 8           # heads
DH = 64          # head dim
B = 4
HH = 56
WW = 56
N = HH * WW      # 3136 query tokens per batch
NQ = N // 2      # 1568 query tokens per core
NKV = 784        # kv tokens per batch (28x28)
MKV = 112        # kv-token chunk (4 rows of 28)
NKV_CH = NKV // MKV  # 7
KC = C // P      # 4 channel chunks
SCALE = (C // NH) ** -0.5  # 0.125
EPS = 1e-5
QN = 392         # query chunk per attention block (4 * 392 = 1568)
NQC = NQ // QN   # 4

# fixed fp8 quantization scales for the q path (x ~ N(0,1), q_w ~ N(0, .02^2))
S_X = 40.0
S_W = 2000.0
QDESCALE = 1.0 / (S_X * S_W)

AluOp = mybir.AluOpType
Act = mybir.ActivationFunctionType
PerfMode = mybir.MatmulPerfMode


def build_nc():
    from concourse import bacc
    nc = bacc.Bacc()

    # host-packed inputs: one contiguous [128, ...] DMA each
    xt8_d = nc.dram_tensor("xt8", [P, KC, NQ], FP8, kind="ExternalInput")
    qw8_d = nc.dram_tensor("qw8", [P, 2, 2, C], FP8, kind="ExternalInput")
    srw_d = nc.dram_tensor("srw", [P, 16, C], BF16, kind="ExternalInput")
    vtA_d = nc.dram_tensor("vtA", [P, 16, 448], BF16, kind="ExternalInput")
    vtB_d = nc.dram_tensor("vtB", [P, 16, NKV - 448], BF16, kind="ExternalInput")
    kvw_d = nc.dram_tensor("kv_w", [P, KC, 2 * C], BF16, kind="ExternalInput")
    projw_d = nc.dram_tensor("proj_w", [P, KC, C], BF16, kind="ExternalInput")
    bias_d = nc.dram_tensor("bias", [1, 3 * C], F32, kind="ExternalInput")
    kvbk_d = nc.dram_tensor("kv_bk", [P, KC], F32, kind="ExternalInput")
    y_d = nc.dram_tensor("y", [NQ, C], F32, kind="ExternalOutput")

    with TileContext(nc) as tc:
        from contextlib import ExitStack

        with ExitStack() as ctx:
            const = ctx.enter_context(tc.tile_pool(name="const", bufs=1))

            # ---- consolidated input DMAs: q path first (unblocks qproj),
            # then conv inputs; proj weights last ----
            xt8_sb = const.tile([P, KC, NQ], FP8, tag="xt8", name="xt8")
            nc.sync.dma_start(out=xt8_sb, in_=xt8_d[:])
            qw8_sb = const.tile([P, 2, 2, C], FP8, tag="qw8", name="qw8")
            nc.sync.dma_start(out=qw8_sb, in_=qw8_d[:])
            bias_row = const.tile([1, 3 * C], F32, tag="biasrow", name="biasrow")
            nc.sync.dma_start(out=bias_row, in_=bias_d[:])
            srw_sb = const.tile([P, 16, C], BF16, tag="srw", name="srw")
            nc.sync.dma_start(out=srw_sb, in_=srw_d[:])
            vt_sb = const.tile([P, 16, NKV], BF16, tag="vt", name="vt")
            nc.sync.dma_start(out=vt_sb[:, :, 0:448], in_=vtA_d[:])
            nc.sync.dma_start(out=vt_sb[:, :, 448:NKV], in_=vtB_d[:])
            kvw_sb = const.tile([P, KC, 2 * C], BF16, tag="kvw", name="kvw")
            nc.sync.dma_start(out=kvw_sb, in_=kvw_d[:])
            kbias_sb = const.tile([P, KC], F32, tag="kb", name="kb")
            nc.sync.dma_start(out=kbias_sb, in_=kvbk_d[:])
            projw_sb = const.tile([P, KC, C], BF16, tag="pw", name="pw")
            nc.sync.dma_start(out=projw_sb, in_=projw_d[:])

            ident = const.tile([P, P], BF16, tag="ident", name="ident")
            make_identity(nc, ident)

            zero_ap = const.tile([P, 1], F32, tag="zconst", name="zconst")
            nc.vector.memset(zero_ap, 0.0)
            nc.const_aps.aps[(F32, 0.0)] = zero_ap[:]
            eps_ap = const.tile([P, 1], F32, tag="epsconst", name="epsconst")
            nc.vector.memset(eps_ap, EPS)

            # Pre-load the gpsimd partition_broadcast ucode library during
            # the startup DMA window (the lazy LOAD_LIB costs ~7us), then
            # broadcast the bias rows across partitions on chip.
            gpw_in = const.tile([1, 8], F32, tag="gpwi", name="gpwi")
            nc.vector.memset(gpw_in, 1.0)
            gpw_out = const.tile([2, 8], F32, tag="gpwo", name="gpwo")
            nc.gpsimd.partition_broadcast(gpw_out[:], gpw_in[:])

            bias_bc = const.tile([P, 3 * C], F32, tag="biasbc", name="biasbc")
            nc.gpsimd.partition_broadcast(bias_bc[:], bias_row[:])
            srb_bc = bias_bc[:, 0:C]
            vvb_bc = bias_bc[:, C:2 * C]
            projb_bc = bias_bc[:, 2 * C:3 * C]

            # persistent activations
            kvT_sb = [const.tile([P, NKV], BF16, tag=f"kvt{j}", name=f"kvt{j}") for j in range(KC)]
            kT_sb = [const.tile([P, NKV], BF16, tag=f"kt{j}", name=f"kt{j}") for j in range(KC)]
            vv_sb = [
                const.tile([P, NH * (DH + 1)], BF16, tag=f"vv{m}", name=f"vv{m}")
                for m in range(NKV_CH)
            ]
            qT_sb = [const.tile([P, NQ], BF16, tag=f"qt{j}", name=f"qt{j}") for j in range(KC)]
            # full-width attention-output accumulator (heads stacked on
            # partitions), consumed by the decoupled output projection
            ot_sb = [const.tile([P, NQ], BF16, tag=f"ot{j}", name=f"ot{j}") for j in range(KC)]

            expp = ctx.enter_context(tc.tile_pool(name="expp", bufs=22))
            early_exps = {}

            # ------- fused phase 1+2+3: conv/LN/transpose/vv + kT + q -------
            with tc.tile_pool(name="w1", bufs=2) as w1, \
                 tc.tile_pool(name="ps_cv", bufs=2, space="PSUM") as ps_cv, \
                 tc.tile_pool(name="ps_tp", bufs=2, space="PSUM") as ps_tp, \
                 tc.tile_pool(name="ps_e", bufs=2, space="PSUM") as ps_e, \
                 tc.tile_pool(name="ps_mm", bufs=2, space="PSUM") as ps_mm:
                xn_tiles = {}

                def conv_ln(m):
                    conv_ps = ps_cv.tile([P, C], F32, tag="conv", name="conv")
                    nmm = 0
                    for idx in range(16):
                        nc.tensor.matmul(
                            conv_ps[:MKV, :],
                            vt_sb[:, idx, m * MKV:(m + 1) * MKV],
                            srw_sb[:, idx, :],
                            start=(nmm == 0),
                            stop=(nmm == 15),
                        )
                        nmm += 1
                    # LayerNorm over the free dim (channels)
                    x_c = w1.tile([P, C], F32, tag="lnx", name="lnx")
                    nc.vector.tensor_tensor(
                        x_c[:MKV], conv_ps[:MKV], srb_bc[:MKV], AluOp.add
                    )
                    sums = w1.tile([P, 1], F32, tag="lnsum", name="lnsum")
                    nc.vector.reduce_sum(
                        out=sums[:MKV], in_=x_c[:MKV], axis=mybir.AxisListType.X
                    )
                    mu = w1.tile([P, 1], F32, tag="lnmu", name="lnmu")
                    nc.vector.tensor_scalar_mul(mu[:MKV], sums[:MKV], 1.0 / C)
                    sq_scr = w1.tile([P, C], BF16, tag="lnsq", name="lnsq")
                    sqs = w1.tile([P, 1], F32, tag="lnsqs", name="lnsqs")
                    nc.scalar.activation(
                        sq_scr[:MKV], x_c[:MKV], Act.Square, accum_out=sqs[:MKV]
                    )
                    mu2 = w1.tile([P, 1], F32, tag="lnmu2", name="lnmu2")
                    nc.vector.tensor_tensor(mu2[:MKV], mu[:MKV], mu[:MKV], AluOp.mult)
                    var = w1.tile([P, 1], F32, tag="lnvar", name="lnvar")
                    nc.vector.tensor_scalar(
                        var[:MKV], sqs[:MKV], 1.0 / C, None, AluOp.mult
                    )
                    nc.vector.tensor_tensor(var[:MKV], var[:MKV], mu2[:MKV], AluOp.subtract)
                    # rstd = exp(-0.5 * ln(var + eps)): keeps ACT on the
                    # Ln/Exp/Square table set (Sqrt would force a reload)
                    lnv = w1.tile([P, 1], F32, tag="lnlnv", name="lnlnv")
                    nc.scalar.activation(lnv[:MKV], var[:MKV], Act.Ln, bias=eps_ap[:MKV])
                    rstd = w1.tile([P, 1], F32, tag="lnrstd", name="lnrstd")
                    nc.scalar.activation(rstd[:MKV], lnv[:MKV], Act.Exp, scale=-0.5)
                    xn = w1.tile([P, C], BF16, tag="lnout", name="lnout")
                    nc.vector.tensor_scalar(
                        xn[:MKV], x_c[:MKV], mu[:MKV], rstd[:MKV],
                        AluOp.subtract, AluOp.mult,
                    )
                    xn_tiles[m] = xn

                def tp_chunk(m):
                    # transpose [112, 512] -> kvT chunks [128, 112]
                    xn = xn_tiles.pop(m)
                    for j in range(KC):
                        tp_ps = ps_tp.tile([P, MKV], BF16, tag="tp", name="tp")
                        nc.tensor.transpose(
                            tp_ps[:, :MKV],
                            xn[:MKV, j * P:(j + 1) * P],
                            ident[:MKV, :MKV],
                        )
                        nc.vector.tensor_copy(
                            kvT_sb[j][:, m * MKV:(m + 1) * MKV], tp_ps[:, :MKV]
                        )

                def vv_chunk(m):
                    vv_ps = ps_mm.tile([P, C], F32, tag="mm", name="vvp")
                    for kc in range(KC):
                        nc.tensor.matmul(
                            vv_ps[:MKV, :],
                            kvT_sb[kc][:, m * MKV:(m + 1) * MKV],
                            kvw_sb[:, kc, C:],
                            start=(kc == 0),
                            stop=(kc == KC - 1),
                        )
                    vv_view = vv_sb[m].rearrange("p (h d) -> p h d", d=DH + 1)
                    nc.vector.tensor_tensor(
                        vv_view[:MKV, :, 0:DH],
                        vv_ps[:MKV].rearrange("p (h d) -> p h d", d=DH),
                        vvb_bc.rearrange("p (h d) -> p h d", d=DH)[:MKV],
                        AluOp.add,
                    )
                    nc.vector.memset(vv_view[:MKV, :, DH:DH + 1], 1.0)

                def kt_part(n0, nn):
                    # k^T[:, n0:n0+nn]: [outc, ktok] = kv_w[:, :512]^T @ kv_^T
                    for j in range(KC):
                        kt_ps = ps_mm.tile([P, C], F32, tag="mm", name="ktp")
                        for kc in range(KC):
                            nc.tensor.matmul(
                                kt_ps[:, :nn],
                                kvw_sb[:, kc, j * P:(j + 1) * P],
                                kvT_sb[kc][:, n0:n0 + nn],
                                start=(kc == 0),
                                stop=(kc == KC - 1),
                            )
                        nc.vector.tensor_scalar_add(
                            kT_sb[j][:, n0:n0 + nn], kt_ps[:, :nn],
                            kbias_sb[:, j:j + 1],
                        )

                def sc_early(p, m):
                    # qc0 scores for kv-chunks 0..3 (they only need the first
                    # kT half), emitted inside the fused-phase tail from a
                    # dedicated PSUM pool so ACT starts the exp pipeline
                    # early, before the PE finishes phase 1-3.
                    e = expp.tile([P, 2 * QN], BF16, tag="expt", name="expt")
                    for s, hb in ((0, 0), (1, DH)):
                        scp = ps_e.tile([P, QN], F32, tag="sce", name="sce")
                        nc.tensor.matmul(
                            scp[:MKV, :QN],
                            kT_sb[p][hb:hb + DH, m * MKV:(m + 1) * MKV],
                            qT_sb[p][hb:hb + DH, 0:QN],
                            start=True,
                            stop=True,
                        )
                        nc.scalar.activation(
                            e[:MKV, s * QN:(s + 1) * QN], scp[:MKV, :QN],
                            Act.Exp, scale=SCALE,
                        )
                    early_exps[(p, m)] = e

                def qproj(q0, qn):
                    # fp8 DoubleRow: each matmul contracts 256 channels
                    # (2 k-groups of 128 split along the lhsT/rhs free dim)
                    for j in range(KC):
                        qp_ps = ps_mm.tile([P, C], F32, tag="mm", name="qp")
                        for g in range(2):
                            nc.tensor.matmul(
                                qp_ps[:, :qn],
                                qw8_sb[:, g, :, j * P:(j + 1) * P],
                                xt8_sb[:, 2 * g:2 * g + 2, q0:q0 + qn],
                                start=(g == 0),
                                stop=(g == 1),
                                perf_mode=PerfMode.DoubleRow,
                            )
                        nc.vector.tensor_scalar_mul(
                            qT_sb[j][:, q0:q0 + qn], qp_ps[:, :qn], QDESCALE
                        )

                qproj(0, QN)
                qproj(QN, QN)
                qproj(2 * QN, QN)
                qproj(3 * QN, QN)
                conv_ln(0)
                conv_ln(1)
                tp_chunk(0)
                vv_chunk(0)
                conv_ln(2)
                tp_chunk(1)
                vv_chunk(1)
                conv_ln(3)
                tp_chunk(2)
                vv_chunk(2)
                conv_ln(4)
                tp_chunk(3)
                vv_chunk(3)
                kt_part(0, 448)
                sc_early(0, 0)
                conv_ln(5)
                sc_early(0, 1)
                tp_chunk(4)
                vv_chunk(4)
                sc_early(0, 2)
                conv_ln(6)
                sc_early(0, 3)
                tp_chunk(5)
                vv_chunk(5)
                sc_early(1, 0)
                tp_chunk(6)
                vv_chunk(6)
                sc_early(1, 1)
                kt_part(448, NKV - 448)
                sc_early(1, 2)
                sc_early(1, 3)

            # ---------------- Phase 4: attention + output projection ---------
            # Head pairs (2p, 2p+1) live at partition offsets 0/64 of kT/qT
            # tile p, so the pair's score matmuls hit disjoint PE row groups
            # and run concurrently into the two banks of one PSUM tile; one
            # exp instruction then covers both heads' scores.
            #
            # The 16 (q-chunk, pair) blocks form one flat software-pipelined
            # stream (scores run one block ahead of attn) so no bubble forms
            # at q-chunk boundaries. Sync-engine discipline: dma_start waits
            # execute in-order ON the Sync engine, so every SBUF-shuffle /
            # output DMA is queued and flushed a block later, when its
            # producer has retired.
            with tc.tile_pool(name="w4", bufs=4) as w4, \
                 tc.tile_pool(name="ps_sc", bufs=2, space="PSUM") as ps_sc, \
                 tc.tile_pool(name="ps_at", bufs=3, space="PSUM") as ps_at, \
                 tc.tile_pool(name="ps_pj", bufs=1, space="PSUM") as ps_pj:
                pend_dma = []
                pend_dma_old = []
                pend_proj = []

                def flush_dmas():
                    # two-block deferral: by the time the Sync engine reaches
                    # these, their producers have retired, so the in-order
                    # Sync queue never blocks on a long semaphore wait.
                    while pend_dma_old:
                        o, i = pend_dma_old.pop(0)
                        nc.sync.dma_start(out=o, in_=i)
                    pend_dma_old.extend(pend_dma)
                    del pend_dma[:]

                def do_scores_pair(qi, p):
                    q0 = qi * QN
                    exps = []
                    for m in range(NKV_CH):
                        if qi == 0 and (p, m) in early_exps:
                            exps.append(early_exps.pop((p, m)))
                            continue
                        # two heads at bank-aligned 512-column halves
                        sc2 = ps_sc.tile([P, 1024], F32, tag="sc2", name="sc2")
                        for s, hb in ((0, 0), (1, DH)):
                            nc.tensor.matmul(
                                sc2[:MKV, s * 512:s * 512 + QN],
                                kT_sb[p][hb:hb + DH, m * MKV:(m + 1) * MKV],
                                qT_sb[p][hb:hb + DH, q0:q0 + QN],
                                start=True,
                                stop=True,
                            )
                        e = expp.tile([P, 2 * QN], BF16, tag="expt", name="expt")
                        nc.scalar.activation(
                            e.rearrange("p (s q) -> p s q", s=2)[:MKV],
                            sc2.rearrange("p (s q) -> p s q", s=2)[:MKV, :, 0:QN],
                            Act.Exp, scale=SCALE,
                        )
                        exps.append(e)
                    return exps

                def do_attn(qi, h, exps):
                    q0 = qi * QN
                    s = h % 2
                    jj, hb = h // 2, s * DH
                    at_ps = ps_at.tile([P, QN], F32, tag="at", name="at")
                    for m in range(NKV_CH):
                        nc.tensor.matmul(
                            at_ps[:DH + 1, :QN],
                            vv_sb[m][:MKV, h * (DH + 1):(h + 1) * (DH + 1)],
                            exps[m][:MKV, s * QN:(s + 1) * QN],
                            start=(m == 0),
                            stop=(m == NKV_CH - 1),
                        )
                    den = w4.tile([1, QN], F32, tag="den", name="den")
                    nc.vector.tensor_copy(den[:, :QN], at_ps[DH:DH + 1, :QN])
                    rb = w4.tile([DH, QN], F32, tag="rb", name="rb")
                    nc.gpsimd.partition_broadcast(rb[:, :QN], den[:, :QN])
                    nc.vector.reciprocal_approx_fast(rb[:, :QN], rb[:, :QN])
                    if hb == 0:
                        nc.vector.tensor_tensor(
                            ot_sb[jj][0:DH, q0:q0 + QN], at_ps[0:DH, :QN],
                            rb[:, :QN], AluOp.mult,
                        )
                    else:
                        oddscr = w4.tile([DH, QN], BF16, tag="oddscr", name="oddscr")
                        nc.vector.tensor_tensor(
                            oddscr[:, :QN], at_ps[0:DH, :QN], rb[:, :QN],
                            AluOp.mult,
                        )
                        pend_dma.append((ot_sb[jj][DH:2 * DH, q0:q0 + QN], oddscr[:, :QN]))

                def do_proj(state):
                    (pq0, mqn) = state
                    pj_ps = ps_pj.tile([P, C], F32, tag="pj", name="pj")
                    for j in range(KC):
                        nc.tensor.matmul(
                            pj_ps[:mqn, :],
                            ot_sb[j][:, pq0:pq0 + mqn],
                            projw_sb[:, j, :],
                            start=(j == 0),
                            stop=(j == KC - 1),
                        )
                    yb = w4.tile([P, C], F32, tag="yb", name="yb")
                    nc.vector.tensor_tensor(
                        yb[:mqn], pj_ps[:mqn], projb_bc[:mqn], AluOp.add
                    )
                    pend_dma.append((y_d[pq0:pq0 + mqn, :], yb[:mqn]))

                pend_proj_stage = []
                proj_wm = [0]  # next unprojected query column

                def retire(block):
                    # staged two deep so a q-chunk's projections only pop
                    # after the flush that issues its odd-head shift DMAs.
                    # Up to two pops per retire drains the backlog before the
                    # final block, keeping projections out of the end tail.
                    (qi, pp, ex) = block
                    for _ in range(2):
                        if pend_proj:
                            do_proj(pend_proj.pop(0))
                    pend_proj.extend(pend_proj_stage)
                    del pend_proj_stage[:]
                    do_attn(qi, 2 * pp, ex)
                    do_attn(qi, 2 * pp + 1, ex)
                    if pp == NH // 2 - 1:
                        # q columns up to (qi+1)*QN are now complete: emit
                        # 128-wide projection chunks up to that watermark
                        limit = (qi + 1) * QN
                        while proj_wm[0] < limit:
                            mqn = min(P, limit - proj_wm[0])
                            if mqn < P and limit != NQ:
                                break  # carry ragged tail into next q-chunk
                            pend_proj_stage.append((proj_wm[0], mqn))
                            proj_wm[0] += mqn

                pend = []
                for qi in range(NQC):
                    for pp in range(NH // 2):
                        pend.append((qi, pp, do_scores_pair(qi, pp)))
                        flush_dmas()
                        if len(pend) > 1:
                            retire(pend.pop(0))
                retire(pend.pop(0))
                flush_dmas()
                flush_dmas()
                pend_proj.extend(pend_proj_stage)
                del pend_proj_stage[:]
                while pend_proj:
                    do_proj(pend_proj.pop(0))
                    flush_dmas()
                    flush_dmas()

    nc.finalize()
    return nc


@functools.lru_cache(maxsize=1)
def _get_nc():
    return build_nc()


def _prepare_in_maps(inputs):
    x = np.asarray(inputs["x"], dtype=np.float32)
    v = np.asarray(inputs["v"], dtype=np.float32)
    q_w = np.asarray(inputs["q_w"], dtype=np.float32)
    kv_w = np.asarray(inputs["kv_w"], dtype=np.float32)
    sr_w = np.asarray(inputs["sr_w"], dtype=np.float32)
    sr_b = np.asarray(inputs["sr_b"], dtype=np.float32)
    ln_g = np.asarray(inputs["ln_g"], dtype=np.float32)
    ln_b = np.asarray(inputs["ln_b"], dtype=np.float32)
    proj_w = np.asarray(inputs["proj_w"], dtype=np.float32)
    proj_b = np.asarray(inputs["proj_b"], dtype=np.float32)

    # fold LN affine into the kv projection: kv_w' = g[:,None]*kv_w,
    # kv_b' = b @ kv_w
    kvw_eff = ln_g[:, None] * kv_w
    kvb_eff = (ln_b @ kv_w).astype(np.float32)

    # [p, kc, :] packings
    kvw_pk = np.ascontiguousarray(
        kvw_eff.reshape(KC, P, 2 * C).transpose(1, 0, 2)).astype(NPBF)
    projw_pk = np.ascontiguousarray(
        proj_w.reshape(KC, P, C).transpose(1, 0, 2)).astype(NPBF)
    # sr_w [O, I, kh, kw] -> [p, (di, dj, kc), o]
    srw_pk = np.ascontiguousarray(
        sr_w.transpose(2, 3, 1, 0).reshape(2, 2, KC, P, C)
        .transpose(3, 0, 1, 2, 4).reshape(P, 16, C)).astype(NPBF)
    # q path fp8: qw8 [p, g, i, m] = q_w[g*256 + i*128 + p, m] * S_W
    qw8 = np.ascontiguousarray(
        np.clip(q_w * S_W, -240.0, 240.0)
        .reshape(2, 2, P, C).transpose(2, 0, 1, 3)).astype(NPF8)
    bias_row = np.concatenate([sr_b, kvb_eff[C:], proj_b])[None, :].astype(np.float32)
    kvbk = np.ascontiguousarray(kvb_eff[:C].reshape(KC, P).T).astype(np.float32)

    in_maps = []
    vT_cache = {}
    for core in range(8):
        b, s = core // 2, core % 2
        if b not in vT_cache:
            # [56,56,C] -> [p, (di,dj,kc), 28*28] conv-slice gather
            vb = v[b].reshape(28, 2, 28, 2, C).transpose(1, 3, 4, 0, 2)
            vt = vb.reshape(2, 2, KC, P, NKV).transpose(3, 0, 1, 2, 4).reshape(P, 16, NKV)
            vT_cache[b] = np.ascontiguousarray(vt).astype(NPBF)
        xs = x[b, s * NQ:(s + 1) * NQ, :]  # [1568, 512]
        xt8 = np.ascontiguousarray(
            np.clip(xs.T * S_X, -240.0, 240.0)
            .reshape(KC, P, NQ).transpose(1, 0, 2)).astype(NPF8)
        in_maps.append({
            "xt8": xt8,
            "qw8": qw8,
            "srw": srw_pk,
            "vtA": np.ascontiguousarray(vT_cache[b][:, :, 0:448]),
            "vtB": np.ascontiguousarray(vT_cache[b][:, :, 448:NKV]),
            "kv_w": kvw_pk,
            "proj_w": projw_pk,
            "bias": bias_row,
            "kv_bk": kvbk,
        })

    return in_maps


def _assemble(results):
    out = np.empty((B, N, C), dtype=np.float32)
    for core in range(8):
        b, s = core // 2, core % 2
        out[b, s * NQ:(s + 1) * NQ, :] = results[core]["y"]
    return out


def kernel(**inputs) -> np.ndarray:
    in_maps = _prepare_in_maps(inputs)
    nc = _get_nc()
    res = bass_utils.run_bass_kernel_spmd(nc, in_maps, core_ids=list(range(8)))
    return _assemble(res.results)


if __name__ == "__main__":
    nc = build_nc()
    print("built ok")


# revision 9
# speedup vs baseline: 1.0946x; 1.0946x over previous
"""Trainium2 Bass kernel for PVT-style spatial-reduction cross-attention.

Problem shapes (hardcoded): x [4, 3136, 512], v [4, 3136, 512], h=w=56,
8 heads (dh=64), sr_ratio=2 -> 784 kv tokens, fp32 I/O.

Sharding: 8 cores = 4 batches x 2 query-token halves. Each core computes
the full conv+LN+kv path for its batch (duplicated within the pair) and
attention + output projection for its 1568 query tokens. No collectives.

Layout strategy: activations are kept channel-major ("transposed") on chip
so every matmul contracts over the partition dim. The host supplies x^T and
v^T (layout choice during sharding). Scores are computed transposed
([ktok, qtok]); softmax denominators come from a ones-column appended to
the value matrix; normalization is deferred through the attention-output
matmul and applied via a gpsimd partition-broadcast of 1/denom.

Performance structure (v1 345us -> v2 205us -> this):
 - inputs host-packed into one [128, F] DRAM tensor per logical tensor and
   loaded with ONE dma_start each (~9 total): the Sync engine serializes
   dma_start issue at ~600ns each, so the old ~51-DMA startup was
   issue-bound, not bandwidth-bound.
 - biases come in as single [1, F] rows and are partition-broadcast on
   chip by gpsimd during the startup DMA window.
 - q projection runs in fp8e4 DoubleRow perf mode (2 rows/cycle): host
   quantizes x^T and q_w with fixed scales; descale is folded into the
   PSUM->SBUF evacuation multiply. q-path quantization noise is softmax-
   attenuated (scores shift by <1%), unlike the v path which must stay
   bf16 (attention output is a near-uniform average over 784 tokens, so
   iid per-token noise survives at full relative strength).
 - LayerNorm rstd = exp(-0.5*ln(var+eps)): Ln/Exp/Square share one ACT
   table set, while Sqrt forced ~7 ACT_TABLE_LOADs (~9us) interleaved
   with the exp stream.
 - query chunks are 4x392 (not 512,512,512,32): exp instructions are
   uniform [112, 784]-free; the old 32-wide tail spent ~200ns/instr ACT
   overhead on 56 tiny instructions.
 - phases fused per kv-chunk: conv -> LN -> transpose -> vv projection;
   kT projection in two batched halves; attention head pairs (2p, 2p+1)
   share kT/qT tiles at partition offsets 0/64 so their score matmuls
   occupy disjoint PE row groups and run concurrently; one ACT exp
   instruction covers both heads' scores (the exp stream on the Scalar
   engine is the kernel's second-longest engine load).
 - the 16 (q-chunk, head-pair) blocks form one flat software-pipelined
   stream (scores one block ahead of attention); output projection runs
   from a full-width [128, 1568] accumulator in 13 query chunks of 128,
   decoupled from the 392-query attention chunking.
 - all phase-4 SBUF-shuffle/output DMAs are deferred two pipeline blocks:
   dma_start semaphore waits execute in-order ON the Sync engine, and an
   early-issued DMA whose producer has not retired stalls every later
   cross-engine notify behind it.
"""

import functools
import sys

import numpy as np

try:
    import concourse.bass as bass
except ImportError:  # pragma: no cover
    sys.path.insert(0, "/opt/trn_rl_repo")
    import concourse.bass as bass

import ml_dtypes
from concourse import bass_utils, mybir
from concourse.masks import make_identity
from concourse.tile import TileContext

BF16 = mybir.dt.bfloat16
F32 = mybir.dt.float32
FP8 = mybir.dt.float8e4
NPBF = ml_dtypes.bfloat16
NPF8 = ml_dtypes.float8_e4m3

P = 128
C = 512          # channels
NH = 8           # heads
DH = 64          # head dim
B = 4
HH = 56
WW = 56
N = HH * WW      # 3136 query tokens per batch
NQ = N // 2      # 1568 query tokens per core
NKV = 784        # kv tokens per batch (28x28)
MKV = 112        # kv-token chunk (4 rows of 28)
NKV_CH = NKV // MKV  # 7
KC = C // P      # 4 channel chunks
SCALE = (C // NH) ** -0.5  # 0.125
EPS = 1e-5
QN = 392         # query chunk per attention block (4 * 392 = 1568)
NQC = NQ // QN   # 4

# fixed fp8 quantization scales for the q path (x ~ N(0,1), q_w ~ N(0, .02^2))
S_X = 40.0
S_W = 2000.0
QDESCALE = 1.0 / (S_X * S_W)

AluOp = mybir.AluOpType
Act = mybir.ActivationFunctionType
PerfMode = mybir.MatmulPerfMode


def build_nc():
    from concourse import bacc
    nc = bacc.Bacc()

    # Pin the ACT table to natural_log_exp_and_others (exp+ln+square), so
    # the table-load fixpoint pass sees every activation covered by one
    # set: without this it greedily ping-pongs exp_and_others <->
    # natural_log (15 ACT_TABLE_LOADs, ~19us of Scalar-engine time).
    nc.scalar.add_instruction(mybir.InstLoadActFuncSet(
        name=nc.get_next_instruction_name(), act_func_set_id=6, ins=[], outs=[]))

    # host-packed inputs: one contiguous [128, ...] DMA each; srw/vt are
    # split so the first half of the conv accumulation can start early
    xt8_d = nc.dram_tensor("xt8", [P, KC, NQ], FP8, kind="ExternalInput")
    qw8_d = nc.dram_tensor("qw8", [P, 2, 2, C], FP8, kind="ExternalInput")
    srw0_d = nc.dram_tensor("srw0", [P, 8, C], BF16, kind="ExternalInput")
    srw1_d = nc.dram_tensor("srw1", [P, 8, C], BF16, kind="ExternalInput")
    vtA0_d = nc.dram_tensor("vtA0", [P, 8, 448], BF16, kind="ExternalInput")
    vtA1_d = nc.dram_tensor("vtA1", [P, 8, 448], BF16, kind="ExternalInput")
    vtB0_d = nc.dram_tensor("vtB0", [P, 8, NKV - 448], BF16, kind="ExternalInput")
    vtB1_d = nc.dram_tensor("vtB1", [P, 8, NKV - 448], BF16, kind="ExternalInput")
    kvw_d = nc.dram_tensor("kv_w", [P, KC, 2 * C], BF16, kind="ExternalInput")
    projw_d = nc.dram_tensor("proj_w", [P, KC, C], BF16, kind="ExternalInput")
    bias_d = nc.dram_tensor("bias", [1, 3 * C], F32, kind="ExternalInput")
    kvbk_d = nc.dram_tensor("kv_bk", [P, KC], F32, kind="ExternalInput")
    y_d = nc.dram_tensor("y", [NQ, C], F32, kind="ExternalOutput")

    with TileContext(nc) as tc:
        from contextlib import ExitStack

        with ExitStack() as ctx:
            const = ctx.enter_context(tc.tile_pool(name="const", bufs=1))

            # ---- consolidated input DMAs: q path first (unblocks qproj),
            # then conv inputs; proj weights last ----
            xt8_sb = const.tile([P, KC, NQ], FP8, tag="xt8", name="xt8")
            nc.sync.dma_start(out=xt8_sb, in_=xt8_d[:])
            qw8_sb = const.tile([P, 2, 2, C], FP8, tag="qw8", name="qw8")
            nc.sync.dma_start(out=qw8_sb, in_=qw8_d[:])
            bias_row = const.tile([1, 3 * C], F32, tag="biasrow", name="biasrow")
            nc.sync.dma_start(out=bias_row, in_=bias_d[:])
            srw_sb = const.tile([P, 16, C], BF16, tag="srw", name="srw")
            vt_sb = const.tile([P, 16, NKV], BF16, tag="vt", name="vt")
            nc.sync.dma_start(out=srw_sb[:, 0:8, :], in_=srw0_d[:])
            nc.sync.dma_start(out=vt_sb[:, 0:8, 0:448], in_=vtA0_d[:])
            nc.sync.dma_start(out=srw_sb[:, 8:16, :], in_=srw1_d[:])
            nc.sync.dma_start(out=vt_sb[:, 8:16, 0:448], in_=vtA1_d[:])
            nc.sync.dma_start(out=vt_sb[:, 0:8, 448:NKV], in_=vtB0_d[:])
            nc.sync.dma_start(out=vt_sb[:, 8:16, 448:NKV], in_=vtB1_d[:])
            kvw_sb = const.tile([P, KC, 2 * C], BF16, tag="kvw", name="kvw")
            nc.sync.dma_start(out=kvw_sb, in_=kvw_d[:])
            kbias_sb = const.tile([P, KC], F32, tag="kb", name="kb")
            nc.sync.dma_start(out=kbias_sb, in_=kvbk_d[:])
            projw_sb = const.tile([P, KC, C], BF16, tag="pw", name="pw")
            nc.sync.dma_start(out=projw_sb, in_=projw_d[:])

            ident = const.tile([P, P], BF16, tag="ident", name="ident")
            make_identity(nc, ident)

            zero_ap = const.tile([P, 1], F32, tag="zconst", name="zconst")
            nc.vector.memset(zero_ap, 0.0)
            nc.const_aps.aps[(F32, 0.0)] = zero_ap[:]
            eps_ap = const.tile([P, 1], F32, tag="epsconst", name="epsconst")
            nc.vector.memset(eps_ap, EPS)

            # Pre-load the gpsimd partition_broadcast ucode library during
            # the startup DMA window (the lazy LOAD_LIB costs ~7us), then
            # broadcast the bias rows across partitions on chip.
            gpw_in = const.tile([1, 8], F32, tag="gpwi", name="gpwi")
            nc.vector.memset(gpw_in, 1.0)
            gpw_out = const.tile([2, 8], F32, tag="gpwo", name="gpwo")
            nc.gpsimd.partition_broadcast(gpw_out[:], gpw_in[:])

            bias_bc = const.tile([P, 3 * C], F32, tag="biasbc", name="biasbc")
            nc.gpsimd.partition_broadcast(bias_bc[:], bias_row[:])
            srb_bc = bias_bc[:, 0:C]
            vvb_bc = bias_bc[:, C:2 * C]
            projb_bc = bias_bc[:, 2 * C:3 * C]

            # persistent activations
            kvT_sb = [const.tile([P, NKV], BF16, tag=f"kvt{j}", name=f"kvt{j}") for j in range(KC)]
            kT_sb = [const.tile([P, NKV], BF16, tag=f"kt{j}", name=f"kt{j}") for j in range(KC)]
            vv_sb = [
                const.tile([P, NH * (DH + 1)], BF16, tag=f"vv{m}", name=f"vv{m}")
                for m in range(NKV_CH)
            ]
            qT_sb = [const.tile([P, NQ], BF16, tag=f"qt{j}", name=f"qt{j}") for j in range(KC)]
            # full-width attention-output accumulator (heads stacked on
            # partitions), consumed by the decoupled output projection
            ot_sb = [const.tile([P, NQ], BF16, tag=f"ot{j}", name=f"ot{j}") for j in range(KC)]

            expp = ctx.enter_context(tc.tile_pool(name="expp", bufs=22))
            early_exps = {}

            # ------- fused phase 1+2+3: conv/LN/transpose/vv + kT + q -------
            # ps_cv is shared between conv accumulators and sc_early score
            # tiles (sc_early only runs once conv has <=2 chunks open)
            with tc.tile_pool(name="w1", bufs=2) as w1, \
                 tc.tile_pool(name="ps_cv", bufs=3, space="PSUM") as ps_cv, \
                 tc.tile_pool(name="ps_tp", bufs=2, space="PSUM") as ps_tp, \
                 tc.tile_pool(name="ps_mm", bufs=3, space="PSUM") as ps_mm:
                xn_tiles = {}
                cv_tiles = {}

                def conv_a(m):
                    # first half of the conv accumulation: only needs the
                    # srw0/vtA0-group DMAs, so it starts ~5us earlier
                    conv_ps = ps_cv.tile([P, C], F32, tag="conv", name="conv")
                    cv_tiles[m] = conv_ps
                    for idx in range(8):
                        nc.tensor.matmul(
                            conv_ps[:MKV, :],
                            vt_sb[:, idx, m * MKV:(m + 1) * MKV],
                            srw_sb[:, idx, :],
                            start=(idx == 0),
                            stop=False,
                        )

                def conv_b(m):
                    conv_ps = cv_tiles.pop(m)
                    for idx in range(8, 16):
                        nc.tensor.matmul(
                            conv_ps[:MKV, :],
                            vt_sb[:, idx, m * MKV:(m + 1) * MKV],
                            srw_sb[:, idx, :],
                            start=False,
                            stop=(idx == 15),
                        )
                    # LayerNorm over the free dim (channels)
                    x_c = w1.tile([P, C], F32, tag="lnx", name="lnx")
                    nc.vector.tensor_tensor(
                        x_c[:MKV], conv_ps[:MKV], srb_bc[:MKV], AluOp.add
                    )
                    sums = w1.tile([P, 1], F32, tag="lnsum", name="lnsum")
                    nc.vector.reduce_sum(
                        out=sums[:MKV], in_=x_c[:MKV], axis=mybir.AxisListType.X
                    )
                    mu = w1.tile([P, 1], F32, tag="lnmu", name="lnmu")
                    nc.vector.tensor_scalar_mul(mu[:MKV], sums[:MKV], 1.0 / C)
                    sq_scr = w1.tile([P, C], BF16, tag="lnsq", name="lnsq")
                    sqs = w1.tile([P, 1], F32, tag="lnsqs", name="lnsqs")
                    nc.scalar.activation(
                        sq_scr[:MKV], x_c[:MKV], Act.Square, accum_out=sqs[:MKV]
                    )
                    mu2 = w1.tile([P, 1], F32, tag="lnmu2", name="lnmu2")
                    nc.vector.tensor_tensor(mu2[:MKV], mu[:MKV], mu[:MKV], AluOp.mult)
                    var = w1.tile([P, 1], F32, tag="lnvar", name="lnvar")
                    nc.vector.tensor_scalar(
                        var[:MKV], sqs[:MKV], 1.0 / C, None, AluOp.mult
                    )
                    nc.vector.tensor_tensor(var[:MKV], var[:MKV], mu2[:MKV], AluOp.subtract)
                    # rstd = exp(-0.5 * ln(var + eps)): keeps ACT on the
                    # Ln/Exp/Square table set (Sqrt would force a reload)
                    lnv = w1.tile([P, 1], F32, tag="lnlnv", name="lnlnv")
                    nc.scalar.activation(lnv[:MKV], var[:MKV], Act.Ln, bias=eps_ap[:MKV])
                    rstd = w1.tile([P, 1], F32, tag="lnrstd", name="lnrstd")
                    nc.scalar.activation(rstd[:MKV], lnv[:MKV], Act.Exp, scale=-0.5)
                    xn = w1.tile([P, C], BF16, tag="lnout", name="lnout")
                    nc.vector.tensor_scalar(
                        xn[:MKV], x_c[:MKV], mu[:MKV], rstd[:MKV],
                        AluOp.subtract, AluOp.mult,
                    )
                    xn_tiles[m] = xn

                def tp_chunk(m):
                    # transpose [112, 512] -> kvT chunks [128, 112]
                    xn = xn_tiles.pop(m)
                    for j in range(KC):
                        tp_ps = ps_tp.tile([P, MKV], BF16, tag="tp", name="tp")
                        nc.tensor.transpose(
                            tp_ps[:, :MKV],
                            xn[:MKV, j * P:(j + 1) * P],
                            ident[:MKV, :MKV],
                        )
                        nc.vector.tensor_copy(
                            kvT_sb[j][:, m * MKV:(m + 1) * MKV], tp_ps[:, :MKV]
                        )

                def vv_chunk(m):
                    vv_ps = ps_mm.tile([P, C], F32, tag="mm", name="vvp")
                    for kc in range(KC):
                        nc.tensor.matmul(
                            vv_ps[:MKV, :],
                            kvT_sb[kc][:, m * MKV:(m + 1) * MKV],
                            kvw_sb[:, kc, C:],
                            start=(kc == 0),
                            stop=(kc == KC - 1),
                        )
                    vv_view = vv_sb[m].rearrange("p (h d) -> p h d", d=DH + 1)
                    nc.vector.tensor_tensor(
                        vv_view[:MKV, :, 0:DH],
                        vv_ps[:MKV].rearrange("p (h d) -> p h d", d=DH),
                        vvb_bc.rearrange("p (h d) -> p h d", d=DH)[:MKV],
                        AluOp.add,
                    )
                    nc.vector.memset(vv_view[:MKV, :, DH:DH + 1], 1.0)

                def kt_part(n0, nn):
                    # k^T[:, n0:n0+nn]: [outc, ktok] = kv_w[:, :512]^T @ kv_^T
                    for j in range(KC):
                        kt_ps = ps_mm.tile([P, C], F32, tag="mm", name="ktp")
                        for kc in range(KC):
                            nc.tensor.matmul(
                                kt_ps[:, :nn],
                                kvw_sb[:, kc, j * P:(j + 1) * P],
                                kvT_sb[kc][:, n0:n0 + nn],
                                start=(kc == 0),
                                stop=(kc == KC - 1),
                            )
                        nc.vector.tensor_scalar_add(
                            kT_sb[j][:, n0:n0 + nn], kt_ps[:, :nn],
                            kbias_sb[:, j:j + 1],
                        )

                def sc_early(p, m):
                    # qc0 scores for kv-chunks 0..3 (they only need the first
                    # kT half), emitted inside the fused-phase tail so ACT
                    # starts the exp pipeline early, before the PE finishes
                    # phase 1-3. PSUM comes from the (now quiet) conv pool.
                    e = expp.tile([P, 2 * QN], BF16, tag="expt", name="expt")
                    for s, hb in ((0, 0), (1, DH)):
                        scp = ps_cv.tile([P, QN], F32, tag="conv", name="sce")
                        nc.tensor.matmul(
                            scp[:MKV, :QN],
                            kT_sb[p][hb:hb + DH, m * MKV:(m + 1) * MKV],
                            qT_sb[p][hb:hb + DH, 0:QN],
                            start=True,
                            stop=True,
                        )
                        nc.scalar.activation(
                            e[:MKV, s * QN:(s + 1) * QN], scp[:MKV, :QN],
                            Act.Exp, scale=SCALE,
                        )
                    early_exps[(p, m)] = e

                def qproj(q0, qn):
                    # fp8 DoubleRow: each matmul contracts 256 channels
                    # (2 k-groups of 128 split along the lhsT/rhs free dim)
                    for j in range(KC):
                        qp_ps = ps_mm.tile([P, C], F32, tag="mm", name="qp")
                        for g in range(2):
                            nc.tensor.matmul(
                                qp_ps[:, :qn],
                                qw8_sb[:, g, :, j * P:(j + 1) * P],
                                xt8_sb[:, 2 * g:2 * g + 2, q0:q0 + qn],
                                start=(g == 0),
                                stop=(g == 1),
                                perf_mode=PerfMode.DoubleRow,
                            )
                        nc.vector.tensor_scalar_mul(
                            qT_sb[j][:, q0:q0 + qn], qp_ps[:, :qn], QDESCALE
                        )

                # NOTE: sc_early reuses ps_cv slots, so every conv_b(m) must
                # be emitted before the sc_early that recycles chunk m's slot
                qproj(0, QN)
                qproj(QN, QN)
                qproj(2 * QN, QN)
                qproj(3 * QN, QN)
                conv_a(0)
                conv_a(1)
                conv_a(2)
                conv_b(0)
                conv_a(3)
                conv_b(1)
                tp_chunk(0)
                tp_chunk(1)
                conv_a(4)
                conv_b(2)
                tp_chunk(2)
                vv_chunk(0)
                vv_chunk(1)
                conv_a(5)
                conv_b(3)
                tp_chunk(3)
                vv_chunk(2)
                conv_a(6)
                conv_b(4)
                tp_chunk(4)
                vv_chunk(3)
                kt_part(0, 448)
                conv_b(5)
                tp_chunk(5)
                vv_chunk(4)
                conv_b(6)
                tp_chunk(6)
                vv_chunk(5)
                sc_early(0, 0)
                sc_early(0, 1)
                vv_chunk(6)
                sc_early(0, 2)
                sc_early(0, 3)
                sc_early(1, 0)
                sc_early(1, 1)
                kt_part(448, NKV - 448)
                sc_early(1, 2)
                sc_early(1, 3)

            # ---------------- Phase 4: attention + output projection ---------
            # Head pairs (2p, 2p+1) live at partition offsets 0/64 of kT/qT
            # tile p, so the pair's score matmuls hit disjoint PE row groups
            # and run concurrently into the two banks of one PSUM tile; one
            # exp instruction then covers both heads' scores.
            #
            # The 16 (q-chunk, pair) blocks form one flat software-pipelined
            # stream (scores run one block ahead of attn) so no bubble forms
            # at q-chunk boundaries. Sync-engine discipline: dma_start waits
            # execute in-order ON the Sync engine, so every SBUF-shuffle /
            # output DMA is queued and flushed a block later, when its
            # producer has retired.
            with tc.tile_pool(name="w4", bufs=4) as w4, \
                 tc.tile_pool(name="ps_sc", bufs=2, space="PSUM") as ps_sc, \
                 tc.tile_pool(name="ps_at", bufs=3, space="PSUM") as ps_at, \
                 tc.tile_pool(name="ps_pj", bufs=1, space="PSUM") as ps_pj:
                pend_dma = []
                pend_dma_old = []
                pend_proj = []

                def flush_dmas():
                    # two-block deferral: by the time the Sync engine reaches
                    # these, their producers have retired, so the in-order
                    # Sync queue never blocks on a long semaphore wait.
                    while pend_dma_old:
                        o, i = pend_dma_old.pop(0)
                        nc.sync.dma_start(out=o, in_=i)
                    pend_dma_old.extend(pend_dma)
                    del pend_dma[:]

                def do_scores_pair(qi, p):
                    q0 = qi * QN
                    exps = []
                    for m in range(NKV_CH):
                        if qi == 0 and (p, m) in early_exps:
                            exps.append(early_exps.pop((p, m)))
                            continue
                        # two heads at bank-aligned 512-column halves
                        sc2 = ps_sc.tile([P, 1024], F32, tag="sc2", name="sc2")
                        for s, hb in ((0, 0), (1, DH)):
                            nc.tensor.matmul(
                                sc2[:MKV, s * 512:s * 512 + QN],
                                kT_sb[p][hb:hb + DH, m * MKV:(m + 1) * MKV],
                                qT_sb[p][hb:hb + DH, q0:q0 + QN],
                                start=True,
                                stop=True,
                            )
                        e = expp.tile([P, 2 * QN], BF16, tag="expt", name="expt")
                        nc.scalar.activation(
                            e.rearrange("p (s q) -> p s q", s=2)[:MKV],
                            sc2.rearrange("p (s q) -> p s q", s=2)[:MKV, :, 0:QN],
                            Act.Exp, scale=SCALE,
                        )
                        exps.append(e)
                    return exps

                def do_attn(qi, h, exps):
                    q0 = qi * QN
                    s = h % 2
                    jj, hb = h // 2, s * DH
                    at_ps = ps_at.tile([P, QN], F32, tag="at", name="at")
                    for m in range(NKV_CH):
                        nc.tensor.matmul(
                            at_ps[:DH + 1, :QN],
                            vv_sb[m][:MKV, h * (DH + 1):(h + 1) * (DH + 1)],
                            exps[m][:MKV, s * QN:(s + 1) * QN],
                            start=(m == 0),
                            stop=(m == NKV_CH - 1),
                        )
                    den = w4.tile([1, QN], F32, tag="den", name="den")
                    nc.vector.tensor_copy(den[:, :QN], at_ps[DH:DH + 1, :QN])
                    rb = w4.tile([DH, QN], F32, tag="rb", name="rb")
                    nc.gpsimd.partition_broadcast(rb[:, :QN], den[:, :QN])
                    nc.vector.reciprocal_approx_fast(rb[:, :QN], rb[:, :QN])
                    if hb == 0:
                        nc.vector.tensor_tensor(
                            ot_sb[jj][0:DH, q0:q0 + QN], at_ps[0:DH, :QN],
                            rb[:, :QN], AluOp.mult,
                        )
                    else:
                        oddscr = w4.tile([DH, QN], BF16, tag="oddscr", name="oddscr")
                        nc.vector.tensor_tensor(
                            oddscr[:, :QN], at_ps[0:DH, :QN], rb[:, :QN],
                            AluOp.mult,
                        )
                        pend_dma.append((ot_sb[jj][DH:2 * DH, q0:q0 + QN], oddscr[:, :QN]))

                def do_proj(state):
                    (pq0, mqn) = state
                    pj_ps = ps_pj.tile([P, C], F32, tag="pj", name="pj")
                    for j in range(KC):
                        nc.tensor.matmul(
                            pj_ps[:mqn, :],
                            ot_sb[j][:, pq0:pq0 + mqn],
                            projw_sb[:, j, :],
                            start=(j == 0),
                            stop=(j == KC - 1),
                        )
                    yb = w4.tile([P, C], F32, tag="yb", name="yb")
                    nc.vector.tensor_tensor(
                        yb[:mqn], pj_ps[:mqn], projb_bc[:mqn], AluOp.add
                    )
                    pend_dma.append((y_d[pq0:pq0 + mqn, :], yb[:mqn]))

                pend_proj_stage = []
                proj_wm = [0]  # next unprojected query column

                def retire(block):
                    # staged two deep so a q-chunk's projections only pop
                    # after the flush that issues its odd-head shift DMAs.
                    # Up to two pops per retire drains the backlog before the
                    # final block, keeping projections out of the end tail.
                    (qi, pp, ex) = block
                    for _ in range(2):
                        if pend_proj:
                            do_proj(pend_proj.pop(0))
                    pend_proj.extend(pend_proj_stage)
                    del pend_proj_stage[:]
                    do_attn(qi, 2 * pp, ex)
                    do_attn(qi, 2 * pp + 1, ex)
                    if pp == NH // 2 - 1:
                        # q columns up to (qi+1)*QN are now complete: emit
                        # 128-wide projection chunks up to that watermark
                        limit = (qi + 1) * QN
                        while proj_wm[0] < limit:
                            mqn = min(P, limit - proj_wm[0])
                            if mqn < P and limit != NQ:
                                break  # carry ragged tail into next q-chunk
                            pend_proj_stage.append((proj_wm[0], mqn))
                            proj_wm[0] += mqn

                pend = []
                for qi in range(NQC):
                    for pp in range(NH // 2):
                        pend.append((qi, pp, do_scores_pair(qi, pp)))
                        flush_dmas()
                        if len(pend) > 1:
                            retire(pend.pop(0))
                retire(pend.pop(0))
                flush_dmas()
                flush_dmas()
                pend_proj.extend(pend_proj_stage)
                del pend_proj_stage[:]
                while pend_proj:
                    do_proj(pend_proj.pop(0))
                    flush_dmas()
                    flush_dmas()

    nc.finalize()
    return nc


@functools.lru_cache(maxsize=1)
def _get_nc():
    return build_nc()


def _prepare_in_maps(inputs):
    x = np.asarray(inputs["x"], dtype=np.float32)
    v = np.asarray(inputs["v"], dtype=np.float32)
    q_w = np.asarray(inputs["q_w"], dtype=np.float32)
    kv_w = np.asarray(inputs["kv_w"], dtype=np.float32)
    sr_w = np.asarray(inputs["sr_w"], dtype=np.float32)
    sr_b = np.asarray(inputs["sr_b"], dtype=np.float32)
    ln_g = np.asarray(inputs["ln_g"], dtype=np.float32)
    ln_b = np.asarray(inputs["ln_b"], dtype=np.float32)
    proj_w = np.asarray(inputs["proj_w"], dtype=np.float32)
    proj_b = np.asarray(inputs["proj_b"], dtype=np.float32)

    # fold LN affine into the kv projection: kv_w' = g[:,None]*kv_w,
    # kv_b' = b @ kv_w
    kvw_eff = ln_g[:, None] * kv_w
    kvb_eff = (ln_b @ kv_w).astype(np.float32)

    # [p, kc, :] packings
    kvw_pk = np.ascontiguousarray(
        kvw_eff.reshape(KC, P, 2 * C).transpose(1, 0, 2)).astype(NPBF)
    projw_pk = np.ascontiguousarray(
        proj_w.reshape(KC, P, C).transpose(1, 0, 2)).astype(NPBF)
    # sr_w [O, I, kh, kw] -> [p, (di, dj, kc), o]
    srw_pk = np.ascontiguousarray(
        sr_w.transpose(2, 3, 1, 0).reshape(2, 2, KC, P, C)
        .transpose(3, 0, 1, 2, 4).reshape(P, 16, C)).astype(NPBF)
    # q path fp8: qw8 [p, g, i, m] = q_w[g*256 + i*128 + p, m] * S_W
    qw8 = np.ascontiguousarray(
        np.clip(q_w * S_W, -240.0, 240.0)
        .reshape(2, 2, P, C).transpose(2, 0, 1, 3)).astype(NPF8)
    bias_row = np.concatenate([sr_b, kvb_eff[C:], proj_b])[None, :].astype(np.float32)
    kvbk = np.ascontiguousarray(kvb_eff[:C].reshape(KC, P).T).astype(np.float32)

    in_maps = []
    vT_cache = {}
    for core in range(8):
        b, s = core // 2, core % 2
        if b not in vT_cache:
            # [56,56,C] -> [p, (di,dj,kc), 28*28] conv-slice gather
            vb = v[b].reshape(28, 2, 28, 2, C).transpose(1, 3, 4, 0, 2)
            vt = vb.reshape(2, 2, KC, P, NKV).transpose(3, 0, 1, 2, 4).reshape(P, 16, NKV)
            vT_cache[b] = np.ascontiguousarray(vt).astype(NPBF)
        xs = x[b, s * NQ:(s + 1) * NQ, :]  # [1568, 512]
        xt8 = np.ascontiguousarray(
            np.clip(xs.T * S_X, -240.0, 240.0)
            .reshape(KC, P, NQ).transpose(1, 0, 2)).astype(NPF8)
        vt = vT_cache[b]
        in_maps.append({
            "xt8": xt8,
            "qw8": qw8,
            "srw0": np.ascontiguousarray(srw_pk[:, 0:8, :]),
            "srw1": np.ascontiguousarray(srw_pk[:, 8:16, :]),
            "vtA0": np.ascontiguousarray(vt[:, 0:8, 0:448]),
            "vtA1": np.ascontiguousarray(vt[:, 8:16, 0:448]),
            "vtB0": np.ascontiguousarray(vt[:, 0:8, 448:NKV]),
            "vtB1": np.ascontiguousarray(vt[:, 8:16, 448:NKV]),
            "kv_w": kvw_pk,
            "proj_w": projw_pk,
            "bias": bias_row,
            "kv_bk": kvbk,
        })

    return in_maps


def _assemble(results):
    out = np.empty((B, N, C), dtype=np.float32)
    for core in range(8):
        b, s = core // 2, core % 2
        out[b, s * NQ:(s + 1) * NQ, :] = results[core]["y"]
    return out


def kernel(**inputs) -> np.ndarray:
    in_maps = _prepare_in_maps(inputs)
    nc = _get_nc()
    res = bass_utils.run_bass_kernel_spmd(nc, in_maps, core_ids=list(range(8)))
    return _assemble(res.results)


if __name__ == "__main__":
    nc = build_nc()
    print("built ok")
